# revision 26
# baseline (speedup 1.0000x reference)
"""Trainium2 Bass kernel for nn_Attention_50826642981451.

Math: with HEAD_DIM=2 the whole attention collapses per (batch b, query i,
key j) to
    S_ij = SCALE * r_i * r_j * cos(theta_j - theta_i + phi)
with r = x0/sqrt(x0^2/2 + eps) (signed rms-normed amplitude), theta_l = l*OMEGA.
Writing Q = [SCALE*qr0, SCALE*qr1, C_i], K = [kr0, kr1, -1] (C_i >= row max,
folded into the dot product so exp never overflows):
    S'_ij = Q_i . K_j  <= ~0,   P = softmax(S'),  out[...,1] = P @ (x1*v_w)

Sharding: pure data parallel, (batch, query-half) across the 8 cores; each
core sees all 4096 keys of its batch.

Device pipeline per core (keys on partitions, queries on the free dim):
  - PE:   S'^T tiles [128 keys x 512 queries] via a 3-term bf16 hi/lo split
          (9 contraction rows, logit-exact to ~2e-3). The 3 matmuls of a
          group run CONCURRENTLY in separate PE row groups (tile_position)
          from row-replicated operands; zero-padding those 32-row slabs also
          keeps the HAM activity monitor counting the PE as busy, which is
          what holds the clock at 2.4 GHz instead of the cold 1.2 GHz.
  - ACT:  E = exp(S'), one instruction per 3 PSUM banks (the saturated
          engine: ~64 us of the ~84 us total).
  - PE:   [numer_hi, numer_lo, denom] accumulated over key tiles via a
          K=128 matmul with lhsT = [w_hi, w_lo, 1] bf16.
  - Emission is software-pipelined (next group's S pack precedes this
    group's reduction matmuls) so the PE queue never head-of-line blocks.
  - A dummy-matmul burst during the input DMA wait pre-warms the HAM clock.
Host does the O(L) trig/norm prep (mirroring the reference's fp32 ops, in
float64 on fp32-rounded elementary values) and the final numer/denom divide.
"""

import math

import numpy as np

B, L, NC = 4, 4096, 8
QPC = (B * L) // NC  # query rows per core = 2048
CHUNK = 512  # queries per PSUM tile / matmul free dim
NKT = L // 128  # 32 key tiles per batch
GROUP = 3  # key tiles per ACT exp instruction (3 PSUM banks)

HEAD_DIM = 2
ROPE_PERIOD = 19.0
OMEGA = 2.0 * math.pi / ROPE_PERIOD
PEAK_EPS = 0.3
TARGET_LOGIT_GAP = math.log(10.0)
ATTN_AMPLITUDE = TARGET_LOGIT_GAP / (
    math.cos(OMEGA * PEAK_EPS) - math.cos(OMEGA * (1.0 - PEAK_EPS))
)
QK_NORM_SCALE = math.sqrt(ATTN_AMPLITUDE / math.sqrt(2.0))
SCALE = HEAD_DIM ** (-0.5) * QK_NORM_SCALE**2

_PROG = None
TRACE = False
LAST_RESULT = None  # BassKernelResults of the most recent device run


def _split_bf16(t):
    """float64 array -> (hi, lo) bf16 pair with hi+lo ~ t to ~2^-17 rel."""
    import ml_dtypes

    hi = t.astype(ml_dtypes.bfloat16)
    lo = (t - hi.astype(np.float64)).astype(ml_dtypes.bfloat16)
    return hi, lo


def _host_prep(x, q_phi, v_w):
    """Mirror the reference's fp32 elementary ops, return device operand arrays.

    Returns per-batch:
      q9[b]: [16, L] bf16 rows (3-term hi/lo split of [Q0, Q1, C])
      k9[b]: [16, L] bf16 rows (matching split of [K0, K1, -1])
      w2[b]: [128, NKT, 2] fp32  (lhsT tiles [w_j, 1] with keys on partitions)
    """
    import ml_dtypes

    f32 = np.float32
    x = np.asarray(x, dtype=f32)
    x0, x1 = x[..., 0], x[..., 1]  # (B, L)
    phi = f32(np.asarray(q_phi).reshape(-1)[0])
    cphi, sphi = f32(np.cos(phi)), f32(np.sin(phi))

    # q/k construction + unit rms norm, all in fp32 like the reference
    q0 = x0 * cphi
    q1 = x0 * (-sphi)
    msq_q = (q0 * q0 + q1 * q1) * f32(0.5)
    rs_q = f32(1.0) / np.sqrt(msq_q + f32(1e-6))
    qn0, qn1 = q0 * rs_q, q1 * rs_q
    msq_k = (x0 * x0) * f32(0.5)
    rs_k = f32(1.0) / np.sqrt(msq_k + f32(1e-6))
    kn0 = x0 * rs_k  # kn1 == 0

    theta = np.arange(L, dtype=f32) * f32(OMEGA)
    ct, st = np.cos(theta).astype(f32), np.sin(theta).astype(f32)
    qr0 = qn0 * ct - qn1 * st
    qr1 = qn0 * st + qn1 * ct
    kr0 = kn0 * ct
    kr1 = kn0 * st

    beta = np.float64(f32(SCALE))
    Q0 = qr0.astype(np.float64) * beta
    Q1 = qr1.astype(np.float64) * beta
    K0 = kr0.astype(np.float64)
    K1 = kr1.astype(np.float64)

    # per-row upper bound of the row max: C_i = beta*|r_i|*max_j|r_j|
    rq = np.hypot(qr0.astype(np.float64), qr1.astype(np.float64))
    rk = np.hypot(K0, K1)
    bmax = rk.max(axis=1, keepdims=True)
    C = beta * rq * bmax  # (B, L)

    Q0h, Q0l = _split_bf16(Q0)
    Q1h, Q1l = _split_bf16(Q1)
    Ch, Cl = _split_bf16(C)
    K0h, K0l = _split_bf16(K0)
    K1h, K1l = _split_bf16(K1)

    bf = ml_dtypes.bfloat16
    neg1 = np.full((B, L), -1.0, dtype=bf)
    zero = np.zeros((B, L), dtype=bf)

    # sum over rows = Q0*K0 + Q1*K1 - C   (up to the dropped lo*lo terms)
    q_rows = [Q0h, Q1h, Ch, Q0l, Q1l, Cl, Q0h, Q1h, Ch]
    k_rows = [K0h, K1h, neg1, K0h, K1h, neg1, K0l, K1l, zero]
    q9 = np.zeros((B, 9, L), dtype=bf)
    k9 = np.zeros((B, 9, L), dtype=bf)
    for r, (qr, kr) in enumerate(zip(q_rows, k_rows)):
        q9[:, r, :] = qr
        k9[:, r, :] = kr

    w = (x1 * f32(np.asarray(v_w).reshape(-1)[0])).astype(np.float64)  # (B, L)
    wh, wl = _split_bf16(w)
    w2 = np.zeros((B, 128, NKT, 3), dtype=bf)
    for kt in range(NKT):
        w2[:, :, kt, 0] = wh[:, kt * 128 : (kt + 1) * 128]
        w2[:, :, kt, 1] = wl[:, kt * 128 : (kt + 1) * 128]
    w2[:, :, :, 2] = 1.0
    return q9, k9, w2


def _build():
    import concourse.mybir as mybir
    import concourse.tile as tile
    from concourse import bacc

    f32 = mybir.dt.float32
    bf16 = mybir.dt.bfloat16
    EXP = mybir.ActivationFunctionType.Exp

    nc = bacc.Bacc()
    q9 = nc.dram_tensor("q9", [9, QPC], bf16, kind="ExternalInput")
    k9 = nc.dram_tensor("k9", [9, L], bf16, kind="ExternalInput")
    w2 = nc.dram_tensor("w2", [128, NKT, 3], bf16, kind="ExternalInput")
    out_nd = nc.dram_tensor("out_nd", [3, QPC], f32, kind="ExternalOutput")

    groups = [list(range(g, min(g + GROUP, NKT))) for g in range(0, NKT, GROUP)]

    with tile.TileContext(nc) as tc:
        with (
            tc.tile_pool(name="const", bufs=1) as cpool,
            tc.tile_pool(name="epool", bufs=4) as epool,
            tc.tile_pool(name="spsum", bufs=2, space="PSUM") as spool,
            tc.tile_pool(name="rpsum", bufs=2, space="PSUM") as rpool,
        ):
            # K padded to 128: HAM only counts full-row-coverage matmuls as
            # "PE busy", so K=16 would pin the PE at the cold 1.2 GHz clock.
            q9_sb = cpool.tile([128, QPC], bf16)
            k9_sb = cpool.tile([128, L], bf16)
            w2_sb = cpool.tile([128, NKT, 3], bf16)
            # The 9 real contraction rows are replicated at partition bases
            # 0/32/64 so the 3 S-matmuls of a group run CONCURRENTLY in
            # different PE row groups via tile_position (~3x PE throughput,
            # and the 3/4 row coverage keeps the HAM clock warm).
            H = L // 2
            nc.gpsimd.memset(k9_sb[:, :H], 0.0)
            nc.vector.memset(q9_sb[:], 0.0)
            nc.vector.memset(k9_sb[:, H:], 0.0)
            nc.sync.dma_start(w2_sb[:], w2[:])
            for rp in (0, 32, 64):
                nc.sync.dma_start(q9_sb[rp : rp + 9, :], q9[:])
                nc.gpsimd.dma_start(k9_sb[rp : rp + 9, :H], k9[:, :H])
            for rp in (0, 32, 64):
                nc.sync.dma_start(k9_sb[rp : rp + 9, H:], k9[:, H:])

            # Pre-warm the PE HAM clock during the input-DMA wait: ~45 dummy
            # full-row matmuls on the (tiny, early-arriving) w2 tile push the
            # activity window past the un-throttle threshold so the first real
            # matmuls already run at 2.4 GHz.


            # Software-pipelined emission: each step issues the NEXT group's
            # S-matmul pack before this group's reduction matmuls, so the PE
            # queue never head-of-line-blocks on the exp at group boundaries.
            seq = [(c, g) for c in range(QPC // CHUNK) for g in range(len(groups))]
            s_tiles = {}
            reds = {}

            def emit_spack(step):
                c, g = seq[step]
                s_ps = spool.tile(
                    [128, GROUP * CHUNK], f32, tag="s", name=f"s_{step}"
                )
                s_tiles[step] = s_ps
                if step == 0:
                    # Pre-warm the PE HAM clock during the input-DMA wait with
                    # dummy matmuls on the (tiny, early-arriving) w2 tile, so
                    # the first real matmuls already run at 2.4 GHz.
                    w2flat = w2_sb.rearrange("p a b -> p (a b)")
                    for _ in range(45):
                        nc.tensor.matmul(
                            s_ps[0:96, 0:96],
                            lhsT=w2flat[:, 0:96],
                            rhs=w2flat[:, 0:96],
                            start=True,
                            stop=True,
                        )
                for gi, kt in enumerate(groups[g]):
                    rp = 32 * gi
                    nc.tensor.matmul(
                        s_ps[:, gi * CHUNK : (gi + 1) * CHUNK],
                        lhsT=k9_sb[rp : rp + 32, kt * 128 : (kt + 1) * 128],
                        rhs=q9_sb[rp : rp + 32, c * CHUNK : (c + 1) * CHUNK],
                        start=True,
                        stop=True,
                        tile_position=(rp, 0),
                    )

            emit_spack(0)
            for step, (c, g) in enumerate(seq):
                kts = groups[g]
                gg = len(kts)
                s_ps = s_tiles.pop(step)
                e_sb = epool.tile([128, GROUP * CHUNK], bf16, tag="e")
                nc.scalar.activation(
                    e_sb[:, : gg * CHUNK], s_ps[:, : gg * CHUNK], EXP
                )
                if step + 1 < len(seq):
                    emit_spack(step + 1)
                if g == 0:
                    reds[c] = rpool.tile([3, CHUNK], f32, tag="red", name=f"red_{c}")
                red = reds[c]
                for gi, kt in enumerate(kts):
                    nc.tensor.matmul(
                        red[:],
                        lhsT=w2_sb[:, kt, :],
                        rhs=e_sb[:, gi * CHUNK : (gi + 1) * CHUNK],
                        start=(kt == 0),
                        stop=(kt == NKT - 1),
                        skip_group_check=True,
                    )
                if g == len(groups) - 1:
                    red_sb = epool.tile([3, CHUNK], f32, tag="red_sb")
                    nc.vector.tensor_copy(red_sb[:], red[:])
                    nc.sync.dma_start(
                        out_nd[:, c * CHUNK : (c + 1) * CHUNK], red_sb[:]
                    )

    # Drop same-engine self-waits on compute instructions: each engine executes
    # its queue serially, so a wait on the engine's own completion semaphore for
    # an earlier instruction is implied by program order. Keeping them makes
    # Bacc's event-semaphore splitting emit extra EVENT_SEMAPHORE instructions
    # on the engine queues (~112 ns each) that throttle the ACT steady state.
    _SELF = {
        "InstActivation": "Activation",
        "InstMatmult": "PE",
        "InstLdweights": "PE",
        "InstTensorCopy": None,  # engine-dependent; resolved below
        "InstMemset": None,
    }
    _ENG_PREFIX = {"PE": "PE", "Activation": "Activation", "DVE": "DVE"}
    for f in nc.m.functions:
        for blk in f.blocks:
            for inst in blk.instructions:
                nm = type(inst).__name__
                if nm not in _SELF:
                    continue
                pref = _SELF[nm]
                if pref is None:
                    eng = str(getattr(inst, "engine", ""))
                    pref = _ENG_PREFIX.get(eng.split(".")[-1])
                    if pref is None:
                        continue
                si = getattr(inst, "sync_info", None)
                if si is None or not si.on_wait:
                    continue
                kept = [
                    w
                    for w in si.on_wait
                    if not str(getattr(w, "ant_name", "")).startswith(pref)
                ]
                if len(kept) < len(si.on_wait):
                    si.on_wait = kept
    nc.finalize()
    return nc


def _numpy_fallback(x, mask, q_phi, v_w):
    """Full-precision host path, used only if mask is nonzero."""
    x = np.asarray(x, dtype=np.float32)
    x0, x1 = x[..., 0].astype(np.float64), x[..., 1].astype(np.float64)
    phi = float(np.asarray(q_phi).reshape(-1)[0])
    r_q = x0 / np.sqrt(x0 * x0 / 2.0 + 1e-6)
    theta = np.arange(L, dtype=np.float64) * OMEGA
    Sg = SCALE * (
        r_q[:, :, None]
        * r_q[:, None, :]
        * np.cos(theta[None, :] - theta[:, None] + phi)[None]
    )
    Sg = Sg + np.asarray(mask, dtype=np.float64)[0, 0][None]
    Sg -= Sg.max(axis=-1, keepdims=True)
    E = np.exp(Sg)
    P = E / E.sum(axis=-1, keepdims=True)
    v = x1 * float(np.asarray(v_w).reshape(-1)[0])
    out1 = np.einsum("bqk,bk->bq", P, v)
    out = np.zeros((B, L, 2), dtype=np.float32)
    out[..., 1] = out1.astype(np.float32)
    return out


def kernel(x, mask, q_phi, v_w):
    global _PROG, LAST_RESULT
    x = np.asarray(x)
    mask = np.asarray(mask)
    if mask.any():
        return _numpy_fallback(x, mask, q_phi, v_w)

    q9, k9, w2 = _host_prep(x, q_phi, v_w)

    from concourse.bass_utils import run_bass_kernel_spmd

    if _PROG is None:
        _PROG = _build()

    in_maps = []
    for c in range(NC):
        b, half = c // 2, c % 2
        in_maps.append(
            {
                "q9": np.ascontiguousarray(
                    q9[b][:, half * QPC : (half + 1) * QPC]
                ),
                "k9": np.ascontiguousarray(k9[b]),
                "w2": np.ascontiguousarray(w2[b]),
            }
        )
    res = run_bass_kernel_spmd(
        _PROG, in_maps, core_ids=list(range(NC)), trace=TRACE
    )
    LAST_RESULT = res

    out = np.zeros((B, L, 2), dtype=np.float32)
    for c in range(NC):
        nd = res.results[c]["out_nd"]  # [3, QPC]: numer_hi, numer_lo, denom
        b, half = c // 2, c % 2
        sl = slice(half * QPC, (half + 1) * QPC)
        numer = nd[0].astype(np.float64) + nd[1].astype(np.float64)
        out[b, sl, 1] = (numer / nd[2].astype(np.float64)).astype(np.float32)
    return out


# revision 27
# speedup vs baseline: 1.0125x; 1.0125x over previous
"""Trainium2 Bass kernel for nn_Attention_50826642981451.

Math: with HEAD_DIM=2 the whole attention collapses per (batch b, query i,
key j) to
    S_ij = SCALE * r_i * r_j * cos(theta_j - theta_i + phi)
with r = x0/sqrt(x0^2/2 + eps) (signed rms-normed amplitude), theta_l = l*OMEGA.
Writing Q = [SCALE*qr0, SCALE*qr1, C_i], K = [kr0, kr1, -1] (C_i >= row max,
folded into the dot product so exp never overflows):
    S'_ij = Q_i . K_j  <= ~0,   P = softmax(S'),  out[...,1] = P @ (x1*v_w)

Sharding: pure data parallel, (batch, query-half) across the 8 cores; each
core sees all 4096 keys of its batch.

Device pipeline per core (keys on partitions, queries on the free dim):
  - PE:   S'^T tiles [128 keys x 512 queries] via a 3-term bf16 hi/lo split
          (9 contraction rows, logit-exact to ~2e-3). The 3 matmuls of a
          group run CONCURRENTLY in separate PE row groups (tile_position)
          from row-replicated operands; zero-padding those 32-row slabs also
          keeps the HAM activity monitor counting the PE as busy, which is
          what holds the clock at 2.4 GHz instead of the cold 1.2 GHz.
  - ACT:  E = exp(S'), one instruction per 3 PSUM banks (the saturated
          engine: ~64 us of the ~84 us total).
  - PE:   [numer_hi, numer_lo, denom] accumulated over key tiles via a
          K=128 matmul with lhsT = [w_hi, w_lo, 1] bf16.
  - Emission is software-pipelined (next group's S pack precedes this
    group's reduction matmuls) so the PE queue never head-of-line blocks.
  - A dummy-matmul burst during the input DMA wait pre-warms the HAM clock.
Host does the O(L) trig/norm prep (mirroring the reference's fp32 ops, in
float64 on fp32-rounded elementary values) and the final numer/denom divide.
"""

import math

import numpy as np

B, L, NC = 4, 4096, 8
QPC = (B * L) // NC  # query rows per core = 2048
CHUNK = 512  # queries per PSUM tile / matmul free dim
NKT = L // 128  # 32 key tiles per batch
GROUP = 3  # key tiles per ACT exp instruction (3 PSUM banks)

HEAD_DIM = 2
ROPE_PERIOD = 19.0
OMEGA = 2.0 * math.pi / ROPE_PERIOD
PEAK_EPS = 0.3
TARGET_LOGIT_GAP = math.log(10.0)
ATTN_AMPLITUDE = TARGET_LOGIT_GAP / (
    math.cos(OMEGA * PEAK_EPS) - math.cos(OMEGA * (1.0 - PEAK_EPS))
)
QK_NORM_SCALE = math.sqrt(ATTN_AMPLITUDE / math.sqrt(2.0))
SCALE = HEAD_DIM ** (-0.5) * QK_NORM_SCALE**2

_PROG = None
TRACE = False
LAST_RESULT = None  # BassKernelResults of the most recent device run


def _split_bf16(t):
    """float64 array -> (hi, lo) bf16 pair with hi+lo ~ t to ~2^-17 rel."""
    import ml_dtypes

    hi = t.astype(ml_dtypes.bfloat16)
    lo = (t - hi.astype(np.float64)).astype(ml_dtypes.bfloat16)
    return hi, lo


def _host_prep(x, q_phi, v_w):
    """Mirror the reference's fp32 elementary ops, return device operand arrays.

    Returns per-batch:
      q9[b]: [16, L] bf16 rows (3-term hi/lo split of [Q0, Q1, C])
      k9[b]: [16, L] bf16 rows (matching split of [K0, K1, -1])
      w2[b]: [128, NKT, 2] fp32  (lhsT tiles [w_j, 1] with keys on partitions)
    """
    import ml_dtypes

    f32 = np.float32
    x = np.asarray(x, dtype=f32)
    x0, x1 = x[..., 0], x[..., 1]  # (B, L)
    phi = f32(np.asarray(q_phi).reshape(-1)[0])
    cphi, sphi = f32(np.cos(phi)), f32(np.sin(phi))

    # q/k construction + unit rms norm, all in fp32 like the reference
    q0 = x0 * cphi
    q1 = x0 * (-sphi)
    msq_q = (q0 * q0 + q1 * q1) * f32(0.5)
    rs_q = f32(1.0) / np.sqrt(msq_q + f32(1e-6))
    qn0, qn1 = q0 * rs_q, q1 * rs_q
    msq_k = (x0 * x0) * f32(0.5)
    rs_k = f32(1.0) / np.sqrt(msq_k + f32(1e-6))
    kn0 = x0 * rs_k  # kn1 == 0

    theta = np.arange(L, dtype=f32) * f32(OMEGA)
    ct, st = np.cos(theta).astype(f32), np.sin(theta).astype(f32)
    qr0 = qn0 * ct - qn1 * st
    qr1 = qn0 * st + qn1 * ct
    kr0 = kn0 * ct
    kr1 = kn0 * st

    beta = np.float64(f32(SCALE))
    Q0 = qr0.astype(np.float64) * beta
    Q1 = qr1.astype(np.float64) * beta
    K0 = kr0.astype(np.float64)
    K1 = kr1.astype(np.float64)

    # per-row upper bound of the row max: C_i = beta*|r_i|*max_j|r_j|
    rq = np.hypot(qr0.astype(np.float64), qr1.astype(np.float64))
    rk = np.hypot(K0, K1)
    bmax = rk.max(axis=1, keepdims=True)
    C = beta * rq * bmax  # (B, L)

    Q0h, Q0l = _split_bf16(Q0)
    Q1h, Q1l = _split_bf16(Q1)
    Ch, Cl = _split_bf16(C)
    K0h, K0l = _split_bf16(K0)
    K1h, K1l = _split_bf16(K1)

    bf = ml_dtypes.bfloat16
    neg1 = np.full((B, L), -1.0, dtype=bf)
    zero = np.zeros((B, L), dtype=bf)

    # sum over rows = Q0*K0 + Q1*K1 - C   (up to the dropped lo*lo terms)
    q_rows = [Q0h, Q1h, Ch, Q0l, Q1l, Cl, Q0h, Q1h, Ch]
    k_rows = [K0h, K1h, neg1, K0h, K1h, neg1, K0l, K1l, zero]
    q9 = np.zeros((B, 9, L), dtype=bf)
    k9 = np.zeros((B, 9, L), dtype=bf)
    for r, (qr, kr) in enumerate(zip(q_rows, k_rows)):
        q9[:, r, :] = qr
        k9[:, r, :] = kr

    w = (x1 * f32(np.asarray(v_w).reshape(-1)[0])).astype(np.float64)  # (B, L)
    wh, wl = _split_bf16(w)
    w2 = np.zeros((B, 128, NKT, 3), dtype=bf)
    for kt in range(NKT):
        w2[:, :, kt, 0] = wh[:, kt * 128 : (kt + 1) * 128]
        w2[:, :, kt, 1] = wl[:, kt * 128 : (kt + 1) * 128]
    w2[:, :, :, 2] = 1.0
    return q9, k9, w2


def _build():
    import concourse.mybir as mybir
    import concourse.tile as tile
    from concourse import bacc

    f32 = mybir.dt.float32
    bf16 = mybir.dt.bfloat16
    EXP = mybir.ActivationFunctionType.Exp

    nc = bacc.Bacc()
    q9 = nc.dram_tensor("q9", [9, QPC], bf16, kind="ExternalInput")
    k9 = nc.dram_tensor("k9", [9, L], bf16, kind="ExternalInput")
    w2 = nc.dram_tensor("w2", [128, NKT, 3], bf16, kind="ExternalInput")
    out_nd = nc.dram_tensor("out_nd", [3, QPC], f32, kind="ExternalOutput")

    groups = [list(range(g, min(g + GROUP, NKT))) for g in range(0, NKT, GROUP)]

    with tile.TileContext(nc) as tc:
        with (
            tc.tile_pool(name="const", bufs=1) as cpool,
            tc.tile_pool(name="epool", bufs=4) as epool,
            tc.tile_pool(name="spsum", bufs=2, space="PSUM") as spool,
            tc.tile_pool(name="rpsum", bufs=2, space="PSUM") as rpool,
        ):
            # K padded to 128: HAM only counts full-row-coverage matmuls as
            # "PE busy", so K=16 would pin the PE at the cold 1.2 GHz clock.
            q9_sb = cpool.tile([128, QPC], bf16)
            k9_sb = cpool.tile([128, L], bf16)
            w2_sb = cpool.tile([128, NKT, 3], bf16)
            # The 9 real contraction rows are replicated at partition bases
            # 0/32/64 so the 3 S-matmuls of a group run CONCURRENTLY in
            # different PE row groups via tile_position (~3x PE throughput,
            # and the 3/4 row coverage keeps the HAM clock warm).
            # Pre-warm source with no DMA dependency: ready ~0.1 us in.
            warm_src = cpool.tile([128, 96], bf16)
            nc.gpsimd.memset(warm_src[:], 1.0)
            # Staged k9 fill: the first matmuls gate only on a narrow first
            # piece (cols 0:512) instead of a whole half.
            P0, H = 512, L // 2
            nc.gpsimd.memset(k9_sb[:, :P0], 0.0)
            for rp in (0, 32, 64):
                nc.gpsimd.dma_start(k9_sb[rp : rp + 9, :P0], k9[:, :P0])
            nc.gpsimd.memset(k9_sb[:, P0:H], 0.0)
            for rp in (0, 32, 64):
                nc.gpsimd.dma_start(k9_sb[rp : rp + 9, P0:H], k9[:, P0:H])
            nc.vector.memset(q9_sb[:], 0.0)
            nc.vector.memset(k9_sb[:, H:], 0.0)
            nc.sync.dma_start(w2_sb[:], w2[:])
            for rp in (0, 32, 64):
                nc.sync.dma_start(q9_sb[rp : rp + 9, :], q9[:])
            for rp in (0, 32, 64):
                nc.sync.dma_start(k9_sb[rp : rp + 9, H:], k9[:, H:])

            # Pre-warm the PE HAM clock during the input-DMA wait: ~45 dummy
            # full-row matmuls on the (tiny, early-arriving) w2 tile push the
            # activity window past the un-throttle threshold so the first real
            # matmuls already run at 2.4 GHz.


            # Software-pipelined emission: each step issues the NEXT group's
            # S-matmul pack before this group's reduction matmuls, so the PE
            # queue never head-of-line-blocks on the exp at group boundaries.
            seq = [(c, g) for c in range(QPC // CHUNK) for g in range(len(groups))]
            s_tiles = {}
            reds = {}

            def emit_spack(step):
                c, g = seq[step]
                s_ps = spool.tile(
                    [128, GROUP * CHUNK], f32, tag="s", name=f"s_{step}"
                )
                s_tiles[step] = s_ps
                if step == 0:
                    # Pre-warm the PE HAM clock during the input-DMA wait with
                    # dummy matmuls on a memset-only tile (no DMA dependency),
                    # so the first real matmuls already run at 2.4 GHz.
                    for _ in range(55):
                        nc.tensor.matmul(
                            s_ps[0:96, 0:96],
                            lhsT=warm_src[:, 0:96],
                            rhs=warm_src[:, 0:96],
                            start=True,
                            stop=True,
                        )
                for gi, kt in enumerate(groups[g]):
                    rp = 32 * gi
                    nc.tensor.matmul(
                        s_ps[:, gi * CHUNK : (gi + 1) * CHUNK],
                        lhsT=k9_sb[rp : rp + 32, kt * 128 : (kt + 1) * 128],
                        rhs=q9_sb[rp : rp + 32, c * CHUNK : (c + 1) * CHUNK],
                        start=True,
                        stop=True,
                        tile_position=(rp, 0),
                    )

            emit_spack(0)
            for step, (c, g) in enumerate(seq):
                kts = groups[g]
                gg = len(kts)
                s_ps = s_tiles.pop(step)
                e_sb = epool.tile([128, GROUP * CHUNK], bf16, tag="e")
                nc.scalar.activation(
                    e_sb[:, : gg * CHUNK], s_ps[:, : gg * CHUNK], EXP
                )
                if step + 1 < len(seq):
                    emit_spack(step + 1)
                if g == 0:
                    reds[c] = rpool.tile([3, CHUNK], f32, tag="red", name=f"red_{c}")
                red = reds[c]
                for gi, kt in enumerate(kts):
                    nc.tensor.matmul(
                        red[:],
                        lhsT=w2_sb[:, kt, :],
                        rhs=e_sb[:, gi * CHUNK : (gi + 1) * CHUNK],
                        start=(kt == 0),
                        stop=(kt == NKT - 1),
                        skip_group_check=True,
                    )
                if g == len(groups) - 1:
                    red_sb = epool.tile([3, CHUNK], f32, tag="red_sb")
                    nc.vector.tensor_copy(red_sb[:], red[:])
                    nc.sync.dma_start(
                        out_nd[:, c * CHUNK : (c + 1) * CHUNK], red_sb[:]
                    )

    # Drop same-engine self-waits on compute instructions: each engine executes
    # its queue serially, so a wait on the engine's own completion semaphore for
    # an earlier instruction is implied by program order. Keeping them makes
    # Bacc's event-semaphore splitting emit extra EVENT_SEMAPHORE instructions
    # on the engine queues (~112 ns each) that throttle the ACT steady state.
    _SELF = {
        "InstActivation": "Activation",
        "InstMatmult": "PE",
        "InstLdweights": "PE",
        "InstTensorCopy": None,  # engine-dependent; resolved below
        "InstMemset": None,
    }
    _ENG_PREFIX = {"PE": "PE", "Activation": "Activation", "DVE": "DVE"}
    for f in nc.m.functions:
        for blk in f.blocks:
            for inst in blk.instructions:
                nm = type(inst).__name__
                if nm not in _SELF:
                    continue
                pref = _SELF[nm]
                if pref is None:
                    eng = str(getattr(inst, "engine", ""))
                    pref = _ENG_PREFIX.get(eng.split(".")[-1])
                    if pref is None:
                        continue
                si = getattr(inst, "sync_info", None)
                if si is None or not si.on_wait:
                    continue
                kept = [
                    w
                    for w in si.on_wait
                    if not str(getattr(w, "ant_name", "")).startswith(pref)
                ]
                if len(kept) < len(si.on_wait):
                    si.on_wait = kept
    nc.finalize()
    return nc


def _numpy_fallback(x, mask, q_phi, v_w):
    """Full-precision host path, used only if mask is nonzero."""
    x = np.asarray(x, dtype=np.float32)
    x0, x1 = x[..., 0].astype(np.float64), x[..., 1].astype(np.float64)
    phi = float(np.asarray(q_phi).reshape(-1)[0])
    r_q = x0 / np.sqrt(x0 * x0 / 2.0 + 1e-6)
    theta = np.arange(L, dtype=np.float64) * OMEGA
    Sg = SCALE * (
        r_q[:, :, None]
        * r_q[:, None, :]
        * np.cos(theta[None, :] - theta[:, None] + phi)[None]
    )
    Sg = Sg + np.asarray(mask, dtype=np.float64)[0, 0][None]
    Sg -= Sg.max(axis=-1, keepdims=True)
    E = np.exp(Sg)
    P = E / E.sum(axis=-1, keepdims=True)
    v = x1 * float(np.asarray(v_w).reshape(-1)[0])
    out1 = np.einsum("bqk,bk->bq", P, v)
    out = np.zeros((B, L, 2), dtype=np.float32)
    out[..., 1] = out1.astype(np.float32)
    return out


def kernel(x, mask, q_phi, v_w):
    global _PROG, LAST_RESULT
    x = np.asarray(x)
    mask = np.asarray(mask)
    if mask.any():
        return _numpy_fallback(x, mask, q_phi, v_w)

    q9, k9, w2 = _host_prep(x, q_phi, v_w)

    from concourse.bass_utils import run_bass_kernel_spmd

    if _PROG is None:
        _PROG = _build()

    in_maps = []
    for c in range(NC):
        b, half = c // 2, c % 2
        in_maps.append(
            {
                "q9": np.ascontiguousarray(
                    q9[b][:, half * QPC : (half + 1) * QPC]
                ),
                "k9": np.ascontiguousarray(k9[b]),
                "w2": np.ascontiguousarray(w2[b]),
            }
        )
    res = run_bass_kernel_spmd(
        _PROG, in_maps, core_ids=list(range(NC)), trace=TRACE
    )
    LAST_RESULT = res

    out = np.zeros((B, L, 2), dtype=np.float32)
    for c in range(NC):
        nd = res.results[c]["out_nd"]  # [3, QPC]: numer_hi, numer_lo, denom
        b, half = c // 2, c % 2
        sl = slice(half * QPC, (half + 1) * QPC)
        numer = nd[0].astype(np.float64) + nd[1].astype(np.float64)
        out[b, sl, 1] = (numer / nd[2].astype(np.float64)).astype(np.float32)
    return out


# revision 28
# speedup vs baseline: 1.0400x; 1.0271x over previous
"""Trainium2 Bass kernel for nn_Attention_50826642981451.

Math: with HEAD_DIM=2 the whole attention collapses per (batch b, query i,
key j) to
    S_ij = SCALE * r_i * r_j * cos(theta_j - theta_i + phi)
with r = x0/sqrt(x0^2/2 + eps) (signed rms-normed amplitude), theta_l = l*OMEGA.
Writing Q = [SCALE*qr0, SCALE*qr1, C_i], K = [kr0, kr1, -1] (C_i >= row max,
folded into the dot product so exp never overflows):
    S'_ij = Q_i . K_j  <= ~0,   P = softmax(S'),  out[...,1] = P @ (x1*v_w)

Sharding: pure data parallel, (batch, query-half) across the 8 cores; each
core sees all 4096 keys of its batch.

Device pipeline per core (keys on partitions, queries on the free dim):
  - PE:   S'^T tiles [128 keys x 512 queries] via a 3-term bf16 hi/lo split
          (9 contraction rows, logit-exact to ~2e-3). The 3 matmuls of a
          group run CONCURRENTLY in separate PE row groups (tile_position)
          from row-replicated operands; zero-padding those 32-row slabs also
          keeps the HAM activity monitor counting the PE as busy, which is
          what holds the clock at 2.4 GHz instead of the cold 1.2 GHz.
  - ACT:  E = exp(S'), one instruction per 3 PSUM banks (the saturated
          engine: ~64 us of the ~84 us total).
  - PE:   [numer_hi, numer_lo, denom] accumulated over key tiles via a
          K=128 matmul with lhsT = [w_hi, w_lo, 1] bf16.
  - Emission is software-pipelined (next group's S pack precedes this
    group's reduction matmuls) so the PE queue never head-of-line blocks.
  - A dummy-matmul burst during the input DMA wait pre-warms the HAM clock.
Host does the O(L) trig/norm prep (mirroring the reference's fp32 ops, in
float64 on fp32-rounded elementary values) and the final numer/denom divide.
"""

import math

import numpy as np

B, L, NC = 4, 4096, 8
QPC = (B * L) // NC  # query rows per core = 2048
CHUNK = 512  # queries per PSUM tile / matmul free dim
NKT = L // 128  # 32 key tiles per batch
GROUP = 3  # key tiles per ACT exp instruction (3 PSUM banks)

HEAD_DIM = 2
ROPE_PERIOD = 19.0
OMEGA = 2.0 * math.pi / ROPE_PERIOD
PEAK_EPS = 0.3
TARGET_LOGIT_GAP = math.log(10.0)
ATTN_AMPLITUDE = TARGET_LOGIT_GAP / (
    math.cos(OMEGA * PEAK_EPS) - math.cos(OMEGA * (1.0 - PEAK_EPS))
)
QK_NORM_SCALE = math.sqrt(ATTN_AMPLITUDE / math.sqrt(2.0))
SCALE = HEAD_DIM ** (-0.5) * QK_NORM_SCALE**2

_PROG = None
TRACE = False
LAST_RESULT = None  # BassKernelResults of the most recent device run


def _split_bf16(t):
    """float64 array -> (hi, lo) bf16 pair with hi+lo ~ t to ~2^-17 rel."""
    import ml_dtypes

    hi = t.astype(ml_dtypes.bfloat16)
    lo = (t - hi.astype(np.float64)).astype(ml_dtypes.bfloat16)
    return hi, lo


def _host_prep(x, q_phi, v_w):
    """Mirror the reference's fp32 elementary ops, return device operand arrays.

    Returns per-batch:
      q9[b]: [16, L] bf16 rows (3-term hi/lo split of [Q0, Q1, C])
      k9[b]: [16, L] bf16 rows (matching split of [K0, K1, -1])
      w2[b]: [128, NKT, 2] fp32  (lhsT tiles [w_j, 1] with keys on partitions)
    """
    import ml_dtypes

    f32 = np.float32
    x = np.asarray(x, dtype=f32)
    x0, x1 = x[..., 0], x[..., 1]  # (B, L)
    phi = f32(np.asarray(q_phi).reshape(-1)[0])
    cphi, sphi = f32(np.cos(phi)), f32(np.sin(phi))

    # q/k construction + unit rms norm, all in fp32 like the reference
    q0 = x0 * cphi
    q1 = x0 * (-sphi)
    msq_q = (q0 * q0 + q1 * q1) * f32(0.5)
    rs_q = f32(1.0) / np.sqrt(msq_q + f32(1e-6))
    qn0, qn1 = q0 * rs_q, q1 * rs_q
    msq_k = (x0 * x0) * f32(0.5)
    rs_k = f32(1.0) / np.sqrt(msq_k + f32(1e-6))
    kn0 = x0 * rs_k  # kn1 == 0

    theta = np.arange(L, dtype=f32) * f32(OMEGA)
    ct, st = np.cos(theta).astype(f32), np.sin(theta).astype(f32)
    qr0 = qn0 * ct - qn1 * st
    qr1 = qn0 * st + qn1 * ct
    kr0 = kn0 * ct
    kr1 = kn0 * st

    beta = np.float64(f32(SCALE))
    Q0 = qr0.astype(np.float64) * beta
    Q1 = qr1.astype(np.float64) * beta
    K0 = kr0.astype(np.float64)
    K1 = kr1.astype(np.float64)

    # per-row upper bound of the row max: C_i = beta*|r_i|*max_j|r_j|
    rq = np.hypot(qr0.astype(np.float64), qr1.astype(np.float64))
    rk = np.hypot(K0, K1)
    bmax = rk.max(axis=1, keepdims=True)
    C = beta * rq * bmax  # (B, L)

    Q0h, Q0l = _split_bf16(Q0)
    Q1h, Q1l = _split_bf16(Q1)
    Ch, Cl = _split_bf16(C)
    K0h, K0l = _split_bf16(K0)
    K1h, K1l = _split_bf16(K1)

    bf = ml_dtypes.bfloat16
    neg1 = np.full((B, L), -1.0, dtype=bf)
    zero = np.zeros((B, L), dtype=bf)

    # sum over rows = Q0*K0 + Q1*K1 - C   (up to the dropped lo*lo terms)
    q_rows = [Q0h, Q1h, Ch, Q0l, Q1l, Cl, Q0h, Q1h, Ch]
    k_rows = [K0h, K1h, neg1, K0h, K1h, neg1, K0l, K1l, zero]
    q9 = np.zeros((B, 9, L), dtype=bf)
    k9 = np.zeros((B, 9, L), dtype=bf)
    for r, (qr, kr) in enumerate(zip(q_rows, k_rows)):
        q9[:, r, :] = qr
        k9[:, r, :] = kr

    w = (x1 * f32(np.asarray(v_w).reshape(-1)[0])).astype(np.float64)  # (B, L)
    wh, wl = _split_bf16(w)
    w2 = np.zeros((B, 128, NKT, 3), dtype=bf)
    for kt in range(NKT):
        w2[:, :, kt, 0] = wh[:, kt * 128 : (kt + 1) * 128]
        w2[:, :, kt, 1] = wl[:, kt * 128 : (kt + 1) * 128]
    w2[:, :, :, 2] = 1.0
    return q9, k9, w2


def _build():
    import concourse.mybir as mybir
    import concourse.tile as tile
    from concourse import bacc

    f32 = mybir.dt.float32
    bf16 = mybir.dt.bfloat16
    EXP = mybir.ActivationFunctionType.Exp

    nc = bacc.Bacc()
    q9 = nc.dram_tensor("q9", [9, QPC], bf16, kind="ExternalInput")
    k9 = nc.dram_tensor("k9", [9, L], bf16, kind="ExternalInput")
    q9h = nc.dram_tensor("q9h", [128, CHUNK], bf16, kind="ExternalInput")
    k9h = nc.dram_tensor("k9h", [128, 512], bf16, kind="ExternalInput")
    w2 = nc.dram_tensor("w2", [128, NKT, 3], bf16, kind="ExternalInput")
    out_nd = nc.dram_tensor("out_nd", [3, QPC], f32, kind="ExternalOutput")

    groups = [list(range(g, min(g + GROUP, NKT))) for g in range(0, NKT, GROUP)]

    with tile.TileContext(nc) as tc:
        with (
            tc.tile_pool(name="const", bufs=1) as cpool,
            tc.tile_pool(name="epool", bufs=4) as epool,
            tc.tile_pool(name="spsum", bufs=2, space="PSUM") as spool,
            tc.tile_pool(name="rpsum", bufs=2, space="PSUM") as rpool,
        ):
            # K padded to 128: HAM only counts full-row-coverage matmuls as
            # "PE busy", so K=16 would pin the PE at the cold 1.2 GHz clock.
            q9_sb = cpool.tile([128, QPC], bf16)
            k9_sb = cpool.tile([128, L], bf16)
            w2_sb = cpool.tile([128, NKT, 3], bf16)
            # The 9 real contraction rows are replicated at partition bases
            # 0/32/64 so the 3 S-matmuls of a group run CONCURRENTLY in
            # different PE row groups via tile_position (~3x PE throughput,
            # and the 3/4 row coverage keeps the HAM clock warm).
            # Pre-warm source with no DMA dependency: ready ~0.1 us in.
            warm_src = cpool.tile([128, 96], bf16)
            nc.gpsimd.memset(warm_src[:], 1.0)
            # Head blocks arrive pre-replicated and pre-zero-padded from the
            # host, so the first chunk's matmuls gate on two small DMAs with
            # no memset dependency. The rest is zero-filled on device and
            # row-replicated at partition bases 0/32/64 off the critical path.
            P0, H = 512, L // 2
            nc.sync.dma_start(q9_sb[:, 0:CHUNK], q9h[:])
            nc.sync.dma_start(k9_sb[:, 0:P0], k9h[:])
            nc.sync.dma_start(w2_sb[:], w2[:])
            nc.gpsimd.memset(k9_sb[:, P0:H], 0.0)
            for rp in (0, 32, 64):
                nc.gpsimd.dma_start(k9_sb[rp : rp + 9, P0:H], k9[:, P0:H])
            nc.vector.memset(q9_sb[:, CHUNK:], 0.0)
            nc.vector.memset(k9_sb[:, H:], 0.0)
            for rp in (0, 32, 64):
                nc.sync.dma_start(q9_sb[rp : rp + 9, CHUNK:], q9[:, CHUNK:])
            for rp in (0, 32, 64):
                nc.sync.dma_start(k9_sb[rp : rp + 9, H:], k9[:, H:])

            # Software-pipelined emission: each step issues the NEXT group's
            # S-matmul pack before this group's reduction matmuls, so the PE
            # queue never head-of-line-blocks on the exp at group boundaries.
            seq = [(c, g) for c in range(QPC // CHUNK) for g in range(len(groups))]
            s_tiles = {}
            reds = {}

            def emit_spack(step):
                c, g = seq[step]
                s_ps = spool.tile(
                    [128, GROUP * CHUNK], f32, tag="s", name=f"s_{step}"
                )
                s_tiles[step] = s_ps
                if step == 0:
                    # Pre-warm the PE HAM clock during the input-DMA wait with
                    # dummy matmuls on a memset-only tile (no DMA dependency),
                    # so the first real matmuls already run at 2.4 GHz.
                    for _ in range(55):
                        nc.tensor.matmul(
                            s_ps[0:96, 0:96],
                            lhsT=warm_src[:, 0:96],
                            rhs=warm_src[:, 0:96],
                            start=True,
                            stop=True,
                        )
                for gi, kt in enumerate(groups[g]):
                    rp = 32 * gi
                    nc.tensor.matmul(
                        s_ps[:, gi * CHUNK : (gi + 1) * CHUNK],
                        lhsT=k9_sb[rp : rp + 32, kt * 128 : (kt + 1) * 128],
                        rhs=q9_sb[rp : rp + 32, c * CHUNK : (c + 1) * CHUNK],
                        start=True,
                        stop=True,
                        tile_position=(rp, 0),
                    )

            emit_spack(0)
            for step, (c, g) in enumerate(seq):
                kts = groups[g]
                gg = len(kts)
                s_ps = s_tiles.pop(step)
                e_sb = epool.tile([128, GROUP * CHUNK], bf16, tag="e")
                nc.scalar.activation(
                    e_sb[:, : gg * CHUNK], s_ps[:, : gg * CHUNK], EXP
                )
                if step + 1 < len(seq):
                    emit_spack(step + 1)
                if g == 0:
                    reds[c] = rpool.tile([3, CHUNK], f32, tag="red", name=f"red_{c}")
                red = reds[c]
                for gi, kt in enumerate(kts):
                    nc.tensor.matmul(
                        red[:],
                        lhsT=w2_sb[:, kt, :],
                        rhs=e_sb[:, gi * CHUNK : (gi + 1) * CHUNK],
                        start=(kt == 0),
                        stop=(kt == NKT - 1),
                        skip_group_check=True,
                    )
                if g == len(groups) - 1:
                    red_sb = epool.tile([3, CHUNK], f32, tag="red_sb")
                    nc.vector.tensor_copy(red_sb[:], red[:])
                    nc.sync.dma_start(
                        out_nd[:, c * CHUNK : (c + 1) * CHUNK], red_sb[:]
                    )

    # Drop same-engine self-waits on compute instructions: each engine executes
    # its queue serially, so a wait on the engine's own completion semaphore for
    # an earlier instruction is implied by program order. Keeping them makes
    # Bacc's event-semaphore splitting emit extra EVENT_SEMAPHORE instructions
    # on the engine queues (~112 ns each) that throttle the ACT steady state.
    _SELF = {
        "InstActivation": "Activation",
        "InstMatmult": "PE",
        "InstLdweights": "PE",
        "InstTensorCopy": None,  # engine-dependent; resolved below
        "InstMemset": None,
    }
    _ENG_PREFIX = {"PE": "PE", "Activation": "Activation", "DVE": "DVE"}
    for f in nc.m.functions:
        for blk in f.blocks:
            for inst in blk.instructions:
                nm = type(inst).__name__
                if nm not in _SELF:
                    continue
                pref = _SELF[nm]
                if pref is None:
                    eng = str(getattr(inst, "engine", ""))
                    pref = _ENG_PREFIX.get(eng.split(".")[-1])
                    if pref is None:
                        continue
                si = getattr(inst, "sync_info", None)
                if si is None or not si.on_wait:
                    continue
                kept = [
                    w
                    for w in si.on_wait
                    if not str(getattr(w, "ant_name", "")).startswith(pref)
                ]
                if len(kept) < len(si.on_wait):
                    si.on_wait = kept
    nc.finalize()
    return nc


def _numpy_fallback(x, mask, q_phi, v_w):
    """Full-precision host path, used only if mask is nonzero."""
    x = np.asarray(x, dtype=np.float32)
    x0, x1 = x[..., 0].astype(np.float64), x[..., 1].astype(np.float64)
    phi = float(np.asarray(q_phi).reshape(-1)[0])
    r_q = x0 / np.sqrt(x0 * x0 / 2.0 + 1e-6)
    theta = np.arange(L, dtype=np.float64) * OMEGA
    Sg = SCALE * (
        r_q[:, :, None]
        * r_q[:, None, :]
        * np.cos(theta[None, :] - theta[:, None] + phi)[None]
    )
    Sg = Sg + np.asarray(mask, dtype=np.float64)[0, 0][None]
    Sg -= Sg.max(axis=-1, keepdims=True)
    E = np.exp(Sg)
    P = E / E.sum(axis=-1, keepdims=True)
    v = x1 * float(np.asarray(v_w).reshape(-1)[0])
    out1 = np.einsum("bqk,bk->bq", P, v)
    out = np.zeros((B, L, 2), dtype=np.float32)
    out[..., 1] = out1.astype(np.float32)
    return out


def kernel(x, mask, q_phi, v_w):
    global _PROG, LAST_RESULT
    x = np.asarray(x)
    mask = np.asarray(mask)
    if mask.any():
        return _numpy_fallback(x, mask, q_phi, v_w)

    q9, k9, w2 = _host_prep(x, q_phi, v_w)

    from concourse.bass_utils import run_bass_kernel_spmd

    if _PROG is None:
        _PROG = _build()

    import ml_dtypes

    in_maps = []
    for c in range(NC):
        b, half = c // 2, c % 2
        q9c = np.ascontiguousarray(q9[b][:, half * QPC : (half + 1) * QPC])
        q9h = np.zeros((128, CHUNK), dtype=ml_dtypes.bfloat16)
        k9h = np.zeros((128, 512), dtype=ml_dtypes.bfloat16)
        for rp in (0, 32, 64):
            q9h[rp : rp + 9] = q9c[:, :CHUNK]
            k9h[rp : rp + 9] = k9[b][:, :512]
        in_maps.append(
            {
                "q9": q9c,
                "k9": np.ascontiguousarray(k9[b]),
                "w2": np.ascontiguousarray(w2[b]),
                "q9h": q9h,
                "k9h": k9h,
            }
        )
    res = run_bass_kernel_spmd(
        _PROG, in_maps, core_ids=list(range(NC)), trace=TRACE
    )
    LAST_RESULT = res

    out = np.zeros((B, L, 2), dtype=np.float32)
    for c in range(NC):
        nd = res.results[c]["out_nd"]  # [3, QPC]: numer_hi, numer_lo, denom
        b, half = c // 2, c % 2
        sl = slice(half * QPC, (half + 1) * QPC)
        numer = nd[0].astype(np.float64) + nd[1].astype(np.float64)
        out[b, sl, 1] = (numer / nd[2].astype(np.float64)).astype(np.float32)
    return out


# revision 29
# speedup vs baseline: 1.0458x; 1.0056x over previous
"""Trainium2 Bass kernel for nn_Attention_50826642981451.

Math: with HEAD_DIM=2 the whole attention collapses per (batch b, query i,
key j) to
    S_ij = SCALE * r_i * r_j * cos(theta_j - theta_i + phi)
with r = x0/sqrt(x0^2/2 + eps) (signed rms-normed amplitude), theta_l = l*OMEGA.
Writing Q = [SCALE*qr0, SCALE*qr1, C_i], K = [kr0, kr1, -1] (C_i >= row max,
folded into the dot product so exp never overflows):
    S'_ij = Q_i . K_j  <= ~0,   P = softmax(S'),  out[...,1] = P @ (x1*v_w)

Sharding: pure data parallel, (batch, query-half) across the 8 cores; each
core sees all 4096 keys of its batch.

Device pipeline per core (keys on partitions, queries on the free dim):
  - PE:   S'^T tiles [128 keys x 512 queries] via a 3-term bf16 hi/lo split
          (9 contraction rows, logit-exact to ~2e-3). The 3 matmuls of a
          group run CONCURRENTLY in separate PE row groups (tile_position)
          from row-replicated operands; zero-padding those 32-row slabs also
          keeps the HAM activity monitor counting the PE as busy, which is
          what holds the clock at 2.4 GHz instead of the cold 1.2 GHz.
  - ACT:  E = exp(S'), one instruction per 3 PSUM banks (the saturated
          engine: ~64 us of the ~84 us total).
  - PE:   [numer_hi, numer_lo, denom] accumulated over key tiles via a
          K=128 matmul with lhsT = [w_hi, w_lo, 1] bf16.
  - Emission is software-pipelined (next group's S pack precedes this
    group's reduction matmuls) so the PE queue never head-of-line blocks.
  - A dummy-matmul burst during the input DMA wait pre-warms the HAM clock.
Host does the O(L) trig/norm prep (mirroring the reference's fp32 ops, in
float64 on fp32-rounded elementary values) and the final numer/denom divide.
"""

import math

import numpy as np

B, L, NC = 4, 4096, 8
QPC = (B * L) // NC  # query rows per core = 2048
CHUNK = 512  # queries per PSUM tile / matmul free dim
NKT = L // 128  # 32 key tiles per batch
GROUP = 3  # key tiles per ACT exp instruction (3 PSUM banks)

HEAD_DIM = 2
ROPE_PERIOD = 19.0
OMEGA = 2.0 * math.pi / ROPE_PERIOD
PEAK_EPS = 0.3
TARGET_LOGIT_GAP = math.log(10.0)
ATTN_AMPLITUDE = TARGET_LOGIT_GAP / (
    math.cos(OMEGA * PEAK_EPS) - math.cos(OMEGA * (1.0 - PEAK_EPS))
)
QK_NORM_SCALE = math.sqrt(ATTN_AMPLITUDE / math.sqrt(2.0))
SCALE = HEAD_DIM ** (-0.5) * QK_NORM_SCALE**2

_PROG = None
TRACE = False
LAST_RESULT = None  # BassKernelResults of the most recent device run


def _split_bf16(t):
    """float64 array -> (hi, lo) bf16 pair with hi+lo ~ t to ~2^-17 rel."""
    import ml_dtypes

    hi = t.astype(ml_dtypes.bfloat16)
    lo = (t - hi.astype(np.float64)).astype(ml_dtypes.bfloat16)
    return hi, lo


def _host_prep(x, q_phi, v_w):
    """Mirror the reference's fp32 elementary ops, return device operand arrays.

    Returns per-batch:
      q9[b]: [16, L] bf16 rows (3-term hi/lo split of [Q0, Q1, C])
      k9[b]: [16, L] bf16 rows (matching split of [K0, K1, -1])
      w2[b]: [128, NKT, 2] fp32  (lhsT tiles [w_j, 1] with keys on partitions)
    """
    import ml_dtypes

    f32 = np.float32
    x = np.asarray(x, dtype=f32)
    x0, x1 = x[..., 0], x[..., 1]  # (B, L)
    phi = f32(np.asarray(q_phi).reshape(-1)[0])
    cphi, sphi = f32(np.cos(phi)), f32(np.sin(phi))

    # q/k construction + unit rms norm, all in fp32 like the reference
    q0 = x0 * cphi
    q1 = x0 * (-sphi)
    msq_q = (q0 * q0 + q1 * q1) * f32(0.5)
    rs_q = f32(1.0) / np.sqrt(msq_q + f32(1e-6))
    qn0, qn1 = q0 * rs_q, q1 * rs_q
    msq_k = (x0 * x0) * f32(0.5)
    rs_k = f32(1.0) / np.sqrt(msq_k + f32(1e-6))
    kn0 = x0 * rs_k  # kn1 == 0

    theta = np.arange(L, dtype=f32) * f32(OMEGA)
    ct, st = np.cos(theta).astype(f32), np.sin(theta).astype(f32)
    qr0 = qn0 * ct - qn1 * st
    qr1 = qn0 * st + qn1 * ct
    kr0 = kn0 * ct
    kr1 = kn0 * st

    beta = np.float64(f32(SCALE))
    Q0 = qr0.astype(np.float64) * beta
    Q1 = qr1.astype(np.float64) * beta
    K0 = kr0.astype(np.float64)
    K1 = kr1.astype(np.float64)

    # per-row upper bound of the row max: C_i = beta*|r_i|*max_j|r_j|
    rq = np.hypot(qr0.astype(np.float64), qr1.astype(np.float64))
    rk = np.hypot(K0, K1)
    bmax = rk.max(axis=1, keepdims=True)
    C = beta * rq * bmax  # (B, L)

    Q0h, Q0l = _split_bf16(Q0)
    Q1h, Q1l = _split_bf16(Q1)
    Ch, Cl = _split_bf16(C)
    K0h, K0l = _split_bf16(K0)
    K1h, K1l = _split_bf16(K1)

    bf = ml_dtypes.bfloat16
    neg1 = np.full((B, L), -1.0, dtype=bf)
    zero = np.zeros((B, L), dtype=bf)

    # sum over rows = Q0*K0 + Q1*K1 - C   (up to the dropped lo*lo terms)
    q_rows = [Q0h, Q1h, Ch, Q0l, Q1l, Cl, Q0h, Q1h, Ch]
    k_rows = [K0h, K1h, neg1, K0h, K1h, neg1, K0l, K1l, zero]
    q9 = np.zeros((B, 9, L), dtype=bf)
    k9 = np.zeros((B, 9, L), dtype=bf)
    for r, (qr, kr) in enumerate(zip(q_rows, k_rows)):
        q9[:, r, :] = qr
        k9[:, r, :] = kr

    w = (x1 * f32(np.asarray(v_w).reshape(-1)[0])).astype(np.float64)  # (B, L)
    wh, wl = _split_bf16(w)
    w2 = np.zeros((B, 128, NKT, 3), dtype=bf)
    for kt in range(NKT):
        w2[:, :, kt, 0] = wh[:, kt * 128 : (kt + 1) * 128]
        w2[:, :, kt, 1] = wl[:, kt * 128 : (kt + 1) * 128]
    w2[:, :, :, 2] = 1.0
    return q9, k9, w2


def _build():
    import concourse.mybir as mybir
    import concourse.tile as tile
    from concourse import bacc

    f32 = mybir.dt.float32
    bf16 = mybir.dt.bfloat16
    EXP = mybir.ActivationFunctionType.Exp

    nc = bacc.Bacc()
    q9 = nc.dram_tensor("q9", [9, QPC], bf16, kind="ExternalInput")
    k9 = nc.dram_tensor("k9", [9, L], bf16, kind="ExternalInput")
    q9h = nc.dram_tensor("q9h", [128, CHUNK], bf16, kind="ExternalInput")
    k9h = nc.dram_tensor("k9h", [128, 512], bf16, kind="ExternalInput")
    w2 = nc.dram_tensor("w2", [128, NKT, 3], bf16, kind="ExternalInput")
    out_nd = nc.dram_tensor("out_nd", [3, QPC], f32, kind="ExternalOutput")

    groups = [list(range(g, min(g + GROUP, NKT))) for g in range(0, NKT, GROUP)]

    with tile.TileContext(nc) as tc:
        with (
            tc.tile_pool(name="const", bufs=1) as cpool,
            tc.tile_pool(name="epool", bufs=4) as epool,
            tc.tile_pool(name="spsum", bufs=2, space="PSUM") as spool,
            tc.tile_pool(name="rpsum", bufs=2, space="PSUM") as rpool,
        ):
            # K padded to 128: HAM only counts full-row-coverage matmuls as
            # "PE busy", so K=16 would pin the PE at the cold 1.2 GHz clock.
            q9_sb = cpool.tile([128, QPC], bf16)
            k9_sb = cpool.tile([128, L], bf16)
            w2_sb = cpool.tile([128, NKT, 3], bf16)
            # The 9 real contraction rows are replicated at partition bases
            # 0/32/64 so the 3 S-matmuls of a group run CONCURRENTLY in
            # different PE row groups via tile_position (~3x PE throughput,
            # and the 3/4 row coverage keeps the HAM clock warm).
            # Pre-warm source with no DMA dependency: ready ~0.1 us in.
            warm_src = cpool.tile([128, 96], bf16)
            nc.gpsimd.memset(warm_src[:], 1.0)
            # Head blocks arrive pre-replicated and pre-zero-padded from the
            # host, so the first chunk's matmuls gate on two small DMAs with
            # no memset dependency. The rest is zero-filled on device and
            # row-replicated at partition bases 0/32/64 off the critical path.
            P0, H = 512, L // 2
            nc.sync.dma_start(q9_sb[:, 0:CHUNK], q9h[:])
            nc.sync.dma_start(k9_sb[:, 0:P0], k9h[:])
            nc.sync.dma_start(w2_sb[:], w2[:])
            nc.gpsimd.memset(k9_sb[:, P0:H], 0.0)
            for rp in (0, 32, 64):
                nc.gpsimd.dma_start(k9_sb[rp : rp + 9, P0:H], k9[:, P0:H])
            nc.vector.memset(q9_sb[:, CHUNK:], 0.0)
            nc.vector.memset(k9_sb[:, H:], 0.0)
            for rp in (0, 32, 64):
                nc.sync.dma_start(q9_sb[rp : rp + 9, CHUNK:], q9[:, CHUNK:])
            for rp in (0, 32, 64):
                nc.sync.dma_start(k9_sb[rp : rp + 9, H:], k9[:, H:])

            # Software-pipelined emission: each step issues the NEXT group's
            # S-matmul pack before this group's reduction matmuls, so the PE
            # queue never head-of-line-blocks on the exp at group boundaries.
            seq = [(c, g) for c in range(QPC // CHUNK) for g in range(len(groups))]
            s_tiles = {}
            reds = {}

            def emit_spack(step):
                c, g = seq[step]
                s_ps = spool.tile(
                    [128, GROUP * CHUNK], f32, tag="s", name=f"s_{step}"
                )
                s_tiles[step] = s_ps
                if step == 0:
                    # Pre-warm the PE HAM clock during the input-DMA wait with
                    # dummy matmuls on a memset-only tile (no DMA dependency),
                    # so the first real matmuls already run at 2.4 GHz.
                    for _ in range(46):
                        nc.tensor.matmul(
                            s_ps[0:96, 0:96],
                            lhsT=warm_src[:, 0:96],
                            rhs=warm_src[:, 0:96],
                            start=True,
                            stop=True,
                        )
                for gi, kt in enumerate(groups[g]):
                    rp = 32 * gi
                    nc.tensor.matmul(
                        s_ps[:, gi * CHUNK : (gi + 1) * CHUNK],
                        lhsT=k9_sb[rp : rp + 32, kt * 128 : (kt + 1) * 128],
                        rhs=q9_sb[rp : rp + 32, c * CHUNK : (c + 1) * CHUNK],
                        start=True,
                        stop=True,
                        tile_position=(rp, 0),
                    )

            emit_spack(0)
            for step, (c, g) in enumerate(seq):
                kts = groups[g]
                gg = len(kts)
                s_ps = s_tiles.pop(step)
                e_sb = epool.tile([128, GROUP * CHUNK], bf16, tag="e")
                nc.scalar.activation(
                    e_sb[:, : gg * CHUNK], s_ps[:, : gg * CHUNK], EXP
                )
                if step + 1 < len(seq):
                    emit_spack(step + 1)
                if g == 0:
                    reds[c] = rpool.tile([3, CHUNK], f32, tag="red", name=f"red_{c}")
                red = reds[c]
                for gi, kt in enumerate(kts):
                    nc.tensor.matmul(
                        red[:],
                        lhsT=w2_sb[:, kt, :],
                        rhs=e_sb[:, gi * CHUNK : (gi + 1) * CHUNK],
                        start=(kt == 0),
                        stop=(kt == NKT - 1),
                        skip_group_check=True,
                    )
                if g == len(groups) - 1:
                    red_sb = epool.tile([3, CHUNK], f32, tag="red_sb")
                    nc.vector.tensor_copy(red_sb[:], red[:])
                    nc.sync.dma_start(
                        out_nd[:, c * CHUNK : (c + 1) * CHUNK], red_sb[:]
                    )

    # Drop same-engine self-waits on compute instructions: each engine executes
    # its queue serially, so a wait on the engine's own completion semaphore for
    # an earlier instruction is implied by program order. Keeping them makes
    # Bacc's event-semaphore splitting emit extra EVENT_SEMAPHORE instructions
    # on the engine queues (~112 ns each) that throttle the ACT steady state.
    _SELF = {
        "InstActivation": "Activation",
        "InstMatmult": "PE",
        "InstLdweights": "PE",
        "InstTensorCopy": None,  # engine-dependent; resolved below
        "InstMemset": None,
    }
    _ENG_PREFIX = {"PE": "PE", "Activation": "Activation", "DVE": "DVE"}
    for f in nc.m.functions:
        for blk in f.blocks:
            for inst in blk.instructions:
                nm = type(inst).__name__
                if nm not in _SELF:
                    continue
                pref = _SELF[nm]
                if pref is None:
                    eng = str(getattr(inst, "engine", ""))
                    pref = _ENG_PREFIX.get(eng.split(".")[-1])
                    if pref is None:
                        continue
                si = getattr(inst, "sync_info", None)
                if si is None or not si.on_wait:
                    continue
                kept = [
                    w
                    for w in si.on_wait
                    if not str(getattr(w, "ant_name", "")).startswith(pref)
                ]
                if len(kept) < len(si.on_wait):
                    si.on_wait = kept
    nc.finalize()
    return nc


def _numpy_fallback(x, mask, q_phi, v_w):
    """Full-precision host path, used only if mask is nonzero."""
    x = np.asarray(x, dtype=np.float32)
    x0, x1 = x[..., 0].astype(np.float64), x[..., 1].astype(np.float64)
    phi = float(np.asarray(q_phi).reshape(-1)[0])
    r_q = x0 / np.sqrt(x0 * x0 / 2.0 + 1e-6)
    theta = np.arange(L, dtype=np.float64) * OMEGA
    Sg = SCALE * (
        r_q[:, :, None]
        * r_q[:, None, :]
        * np.cos(theta[None, :] - theta[:, None] + phi)[None]
    )
    Sg = Sg + np.asarray(mask, dtype=np.float64)[0, 0][None]
    Sg -= Sg.max(axis=-1, keepdims=True)
    E = np.exp(Sg)
    P = E / E.sum(axis=-1, keepdims=True)
    v = x1 * float(np.asarray(v_w).reshape(-1)[0])
    out1 = np.einsum("bqk,bk->bq", P, v)
    out = np.zeros((B, L, 2), dtype=np.float32)
    out[..., 1] = out1.astype(np.float32)
    return out


def kernel(x, mask, q_phi, v_w):
    global _PROG, LAST_RESULT
    x = np.asarray(x)
    mask = np.asarray(mask)
    if mask.any():
        return _numpy_fallback(x, mask, q_phi, v_w)

    q9, k9, w2 = _host_prep(x, q_phi, v_w)

    from concourse.bass_utils import run_bass_kernel_spmd

    if _PROG is None:
        _PROG = _build()

    import ml_dtypes

    in_maps = []
    for c in range(NC):
        b, half = c // 2, c % 2
        q9c = np.ascontiguousarray(q9[b][:, half * QPC : (half + 1) * QPC])
        q9h = np.zeros((128, CHUNK), dtype=ml_dtypes.bfloat16)
        k9h = np.zeros((128, 512), dtype=ml_dtypes.bfloat16)
        for rp in (0, 32, 64):
            q9h[rp : rp + 9] = q9c[:, :CHUNK]
            k9h[rp : rp + 9] = k9[b][:, :512]
        in_maps.append(
            {
                "q9": q9c,
                "k9": np.ascontiguousarray(k9[b]),
                "w2": np.ascontiguousarray(w2[b]),
                "q9h": q9h,
                "k9h": k9h,
            }
        )
    res = run_bass_kernel_spmd(
        _PROG, in_maps, core_ids=list(range(NC)), trace=TRACE
    )
    LAST_RESULT = res

    out = np.zeros((B, L, 2), dtype=np.float32)
    for c in range(NC):
        nd = res.results[c]["out_nd"]  # [3, QPC]: numer_hi, numer_lo, denom
        b, half = c // 2, c % 2
        sl = slice(half * QPC, (half + 1) * QPC)
        numer = nd[0].astype(np.float64) + nd[1].astype(np.float64)
        out[b, sl, 1] = (numer / nd[2].astype(np.float64)).astype(np.float32)
    return out
